# revision 2
# baseline (speedup 1.0000x reference)
"""Trainium2 Bass kernel v2 for nn_CustomLoss (YOLO CIoU+BCE loss).

Design vs baseline:
 - SWDGE cast-DMA loads fp32 HBM -> fp16 SBUF (half footprint).
 - Geometry intermediates are dense bf16 "plane" tiles [planes, pitch]
   (pitch-padded, 4B-aligned) so DVE tensor_tensor runs 2x, tensor_scalar 4x.
 - abs-max overlap identity: overlap = (pw+tw)/2 - max(|dc|,|dhw|), enclosing
   wh = (pw+tw)/2 + max(...); cdist = dc^2. Kills plo/phi/clo/chi entirely.
 - Division-free anchor argmax via cross-multiplied IoU compares.
 - Selection: one 8-plane GEO tile (in-place anchor-0) + 11 obj/cls channels
   via copy_predicated.
 - BCE: z = t ? p : 1-p (fp16, clamped >=1e-4), masked rows forced to 1.0 so
   ACT Ln(z) with accum_out yields the masked sums for free.
 - Two half-chunks per iteration with per-chunk tiles: two independent op
   chains in flight => engines overlap (DVE on chunk A ciou while ACT/GPSIMD
   run chunk B geometry).
Engine split: ACT = strided copies/scales + Ln/Arctan; GPSIMD = side dense
adds/products + Z construction; DVE = the rest.
"""

import numpy as np

B, A, N, CH = 64, 3, 8400, 15
NCORES = 8
BPC = B // NCORES       # 8 batches per core
SEC = 16                # sections per batch -> 128 partitions
PPART = BPC * SEC
L = N // SEC            # 525 positions per partition row
CHUNKS = ((0, 263), (263, 262))
C11 = 11
EPS = 1e-7

_CACHE = {}


def _build(loop_r=None, level=4, p_bufs=2, t_bufs=1, use_absmax=False,
           act_accum=True):
    import contextlib
    import concourse.tile as tile
    import concourse.mybir as mybir
    from concourse import bacc

    Alu = mybir.AluOpType
    Act = mybir.ActivationFunctionType
    f32 = mybir.dt.float32
    f16 = mybir.dt.float16
    bf16 = mybir.dt.bfloat16
    i16 = mybir.dt.int16

    nc = bacc.Bacc("TRN2", target_bir_lowering=False, debug=False,
                   num_devices=NCORES)
    predL = nc.dram_tensor("predL", [BPC, A, N, CH], f32, kind="ExternalInput").ap()
    targL = nc.dram_tensor("targL", [BPC, N, CH], f32, kind="ExternalInput").ap()
    accO = nc.dram_tensor("acc_out", [PPART, 8], f32, kind="ExternalOutput").ap()

    pre = predL.rearrange("b a (s j) c -> b a s (j c)", s=SEC, j=L)
    tre = targL.rearrange("b (s j) c -> b s (j c)", s=SEC, j=L)

    with tile.TileContext(nc) as tc:
        with (
            tc.tile_pool(name="pP", bufs=p_bufs) as pP,
            tc.tile_pool(name="pT", bufs=t_bufs) as pT,
            tc.tile_pool(name="pW", bufs=1) as pW,
            tc.tile_pool(name="pAcc", bufs=1) as pAcc,
        ):
            ACC = pAcc.tile([PPART, 8], f32)
            ONES = pAcc.tile([PPART, 2], f16)
            nc.vector.memset(ONES[:], 1.0)

            loop_cm = tc.For_i(0, loop_r, 1) if loop_r else contextlib.nullcontext()
            with loop_cm:
              for ck, (off, Lk) in enumerate(CHUNKS):
                PI = (Lk + 1) // 2 * 2  # even pitch
                cb = 4 * ck             # ACC column base for this chunk
                T = pT.tile([PPART, Lk * CH], f16)
                nc.gpsimd.dma_start(T[:], tre[:, :, off * CH:(off + Lk) * CH])
                P = pP.tile([PPART, A * Lk * CH], f16)
                for a in range(A):
                    nc.gpsimd.dma_start(
                        P[:, a * Lk * CH:(a + 1) * Lk * CH],
                        pre[:, a, :, off * CH:(off + Lk) * CH])

                Pr = P[:].rearrange("p (a j c) -> p a c j", a=A, c=CH)
                Tr = T[:].rearrange("p (j c) -> p c j", c=CH)

                # ---- target prep ----
                TWHH2 = pW.tile([PPART, 2 * PI], bf16)
                Mm = pW.tile([PPART, PI], bf16)
                twhh2 = TWHH2[:].rearrange("p (c j) -> p c j", c=2)
                nc.scalar.activation(twhh2[:, :, 0:Lk], Tr[:, 2:4, :], Act.Copy,
                                     scale=0.5)
                nc.scalar.activation(Mm[:, 0:Lk], Tr[:, 4, :], Act.Copy,
                                     accum_out=ACC[:, cb + 3:cb + 4])
                TA4 = pW.tile([PPART, PI], bf16)
                nc.gpsimd.tensor_tensor(TA4[:, 0:Lk], twhh2[:, 0, 0:Lk],
                                        twhh2[:, 1, 0:Lk], Alu.mult)
                INVM = pW.tile([PPART, PI], bf16)
                nc.vector.tensor_scalar(INVM[:, 0:Lk], Mm[:, 0:Lk], -1.0, 1.0,
                                        Alu.mult, Alu.add)
                if level < 1:
                    continue

                # ---- per-anchor geometry ----
                # GEO planes per anchor: 0:2 e2, 2:4 mx, 4:6 pwhh, 6 i4, 7 u4
                GEO = pW.tile([PPART, A * 8 * PI], bf16)
                geo = GEO[:].rearrange("p (a c j) -> p a c j", a=A, c=8)
                pwhh = geo[:, :, 4:6, :]
                nc.scalar.activation(pwhh[:, :, :, 0:Lk], Pr[:, :, 2:4, :],
                                     Act.Copy, scale=0.5)
                txyb = Tr[:, 0:2, :].unsqueeze(1).broadcast_to(
                    [PPART, A, 2, Lk])
                twhhb = twhh2[:, :, 0:Lk].unsqueeze(1).broadcast_to(
                    [PPART, A, 2, Lk])
                nc.vector.tensor_tensor(geo[:, :, 0:2, 0:Lk], Pr[:, :, 0:2, :],
                                        txyb, Alu.subtract)
                F2 = pW.tile([PPART, A * 2 * PI], bf16)
                f2 = F2[:].rearrange("p (a c j) -> p a c j", a=A, c=2)
                nc.vector.tensor_tensor(f2[:, :, :, 0:Lk], pwhh[:, :, :, 0:Lk],
                                        twhhb, Alu.subtract)
                if use_absmax:
                    nc.vector.tensor_tensor(geo[:, :, 2:4, 0:Lk],
                                            geo[:, :, 0:2, 0:Lk],
                                            f2[:, :, :, 0:Lk], Alu.abs_max)
                else:
                    # mx = max(|e|,|f|) = max(max(e,f), -min(e,f))
                    nc.vector.tensor_tensor(geo[:, :, 2:4, 0:Lk],
                                            geo[:, :, 0:2, 0:Lk],
                                            f2[:, :, :, 0:Lk], Alu.max)
                    nc.vector.tensor_tensor(f2[:, :, :, 0:Lk],
                                            geo[:, :, 0:2, 0:Lk],
                                            f2[:, :, :, 0:Lk], Alu.min)
                    nc.vector.tensor_scalar(f2[:, :, :, 0:Lk],
                                            f2[:, :, :, 0:Lk], -1.0, None,
                                            Alu.mult)
                    nc.vector.tensor_tensor(geo[:, :, 2:4, 0:Lk],
                                            geo[:, :, 2:4, 0:Lk],
                                            f2[:, :, :, 0:Lk], Alu.max)
                # f2 <- cd2 = pwhh + twhh
                nc.gpsimd.tensor_tensor(f2[:, :, :, 0:Lk], pwhh[:, :, :, 0:Lk],
                                        twhhb, Alu.add)
                # f2 <- whd = cd2 - mx (in place)
                nc.vector.tensor_tensor(f2[:, :, :, 0:Lk], f2[:, :, :, 0:Lk],
                                        geo[:, :, 2:4, 0:Lk], Alu.subtract)
                # f2 <- o2 = relu(whd)*0.5 (scale folds inside)
                nc.scalar.activation(f2[:, :, :, 0:Lk], f2[:, :, :, 0:Lk],
                                     Act.Relu, scale=0.5)
                # i4 = o2x*o2y
                nc.vector.tensor_tensor(geo[:, :, 6, 0:Lk], f2[:, :, 0, 0:Lk],
                                        f2[:, :, 1, 0:Lk], Alu.mult)
                PA = pW.tile([PPART, A * PI], bf16)
                pa = PA[:].rearrange("p (a j) -> p a j", a=A)
                nc.vector.tensor_tensor(pa[:, :, 0:Lk], pwhh[:, :, 0, 0:Lk],
                                        pwhh[:, :, 1, 0:Lk], Alu.mult)
                ta4b = TA4[:, 0:Lk].unsqueeze(1).broadcast_to([PPART, A, Lk])
                nc.gpsimd.tensor_tensor(geo[:, :, 7, 0:Lk], pa[:, :, 0:Lk],
                                        ta4b, Alu.add)  # s4
                nc.vector.tensor_tensor(geo[:, :, 7, 0:Lk], geo[:, :, 7, 0:Lk],
                                        geo[:, :, 6, 0:Lk],
                                        Alu.subtract)  # u4

                # ---- argmax (cross products, first-max semantics) ----
                CA = pW.tile([PPART, 2 * PI], bf16)
                CB = pW.tile([PPART, 2 * PI], bf16)
                CM = pW.tile([PPART, 2 * PI], bf16)
                GAB = pW.tile([PPART, 2 * PI], bf16)
                car = CA[:].rearrange("p (c j) -> p c j", c=2)
                cbr = CB[:].rearrange("p (c j) -> p c j", c=2)
                cmr = CM[:].rearrange("p (c j) -> p c j", c=2)
                gabr = GAB[:].rearrange("p (c j) -> p c j", c=2)
                i4 = geo[:, :, 6, 0:Lk]
                u4 = geo[:, :, 7, 0:Lk]
                nc.vector.tensor_tensor(car[:, :, 0:Lk], i4[:, 1:3],
                                        u4[:, 0:2], Alu.mult)
                nc.vector.tensor_tensor(cbr[:, :, 0:Lk], i4[:, 0:2],
                                        u4[:, 1:3], Alu.mult)
                nc.vector.tensor_tensor(cmr[:, 0, 0:Lk], i4[:, 2], u4[:, 0],
                                        Alu.mult)
                nc.vector.tensor_tensor(cmr[:, 1, 0:Lk], i4[:, 0], u4[:, 2],
                                        Alu.mult)
                nc.vector.tensor_tensor(gabr[:, :, 0:Lk], car[:, :, 0:Lk],
                                        cbr[:, :, 0:Lk], Alu.is_gt)
                GN = pW.tile([PPART, PI], bf16)
                G20 = pW.tile([PPART, PI], bf16)
                nc.vector.tensor_tensor(GN[:, 0:Lk], car[:, 1, 0:Lk],
                                        cbr[:, 1, 0:Lk], Alu.is_le)
                nc.vector.tensor_tensor(G20[:, 0:Lk], cmr[:, 0, 0:Lk],
                                        cmr[:, 1, 0:Lk], Alu.is_gt)
                W1 = pW.tile([PPART, PI], bf16)
                W2 = pW.tile([PPART, PI], bf16)
                nc.vector.tensor_tensor(W1[:, 0:Lk], gabr[:, 0, 0:Lk],
                                        GN[:, 0:Lk], Alu.mult)
                nc.vector.tensor_tensor(W2[:, 0:Lk], G20[:, 0:Lk],
                                        gabr[:, 1, 0:Lk], Alu.mult)
                if level < 2:
                    continue

                # ---- selection ----
                w1i = W1[:, 0:Lk].bitcast(i16)
                w2i = W2[:, 0:Lk].bitcast(i16)
                w1b8 = w1i.unsqueeze(1).broadcast_to([PPART, 8, Lk])
                w2b8 = w2i.unsqueeze(1).broadcast_to([PPART, 8, Lk])
                nc.vector.copy_predicated(geo[:, 0, :, 0:Lk], w1b8,
                                          geo[:, 1, :, 0:Lk])
                nc.vector.copy_predicated(geo[:, 0, :, 0:Lk], w2b8,
                                          geo[:, 2, :, 0:Lk])
                SELP = pW.tile([PPART, C11 * PI], f16)
                selp = SELP[:].rearrange("p (c j) -> p c j", c=C11)
                nc.scalar.activation(selp[:, :, 0:Lk], Pr[:, 0, 4:CH, :],
                                     Act.Copy)
                w1b11 = w1i.unsqueeze(1).broadcast_to([PPART, C11, Lk])
                w2b11 = w2i.unsqueeze(1).broadcast_to([PPART, C11, Lk])
                nc.vector.copy_predicated(selp[:, :, 0:Lk], w1b11,
                                          Pr[:, 1, 4:CH, :])
                nc.vector.copy_predicated(selp[:, :, 0:Lk], w2b11,
                                          Pr[:, 2, 4:CH, :])
                if level < 3:
                    continue

                # ---- ciou on selected (anchor-0 planes) ----
                se2 = geo[:, 0, 0:2, 0:Lk]
                smx = geo[:, 0, 2:4, 0:Lk]
                spw = geo[:, 0, 4:6, 0:Lk]
                si4 = geo[:, 0, 6, 0:Lk]
                su4 = geo[:, 0, 7, 0:Lk]
                cwh = gabr  # reuse GAB (dead after W1/W2)
                nc.gpsimd.tensor_tensor(cwh[:, :, 0:Lk], spw,
                                        twhh2[:, :, 0:Lk], Alu.add)
                nc.gpsimd.tensor_tensor(cwh[:, :, 0:Lk], cwh[:, :, 0:Lk], smx,
                                        Alu.add)
                sqe = car  # reuse CA
                sqc = cbr  # reuse CB
                nc.scalar.activation(sqe[:, :, 0:Lk], se2, Act.Square)
                nc.scalar.activation(sqc[:, :, 0:Lk], cwh[:, :, 0:Lk],
                                     Act.Square)
                DIAG = pW.tile([PPART, PI], f32)
                nc.vector.scalar_tensor_tensor(DIAG[:, 0:Lk], sqc[:, 0, 0:Lk],
                                               EPS, sqc[:, 1, 0:Lk], Alu.add,
                                               Alu.add)
                nc.vector.reciprocal_approx_fast(DIAG[:, 0:Lk], DIAG[:, 0:Lk])
                CDR = pW.tile([PPART, PI], bf16)
                nc.gpsimd.tensor_tensor(CDR[:, 0:Lk], sqe[:, 0, 0:Lk],
                                        sqe[:, 1, 0:Lk], Alu.add)
                QD = pW.tile([PPART, PI], bf16)
                nc.vector.tensor_tensor(QD[:, 0:Lk], CDR[:, 0:Lk],
                                        DIAG[:, 0:Lk], Alu.mult)
                DEN = pW.tile([PPART, PI], f32)
                nc.vector.tensor_scalar(DEN[:, 0:Lk], su4, EPS, None, Alu.add)
                nc.vector.reciprocal_approx_fast(DEN[:, 0:Lk], DEN[:, 0:Lk])
                UMI = GN  # reuse (dead after W1)
                nc.gpsimd.tensor_tensor(UMI[:, 0:Lk], su4, si4, Alu.subtract)
                OMI = G20  # reuse (dead after W2)
                nc.vector.tensor_tensor(OMI[:, 0:Lk], UMI[:, 0:Lk],
                                        DEN[:, 0:Lk], Alu.mult)
                DIOU = pW.tile([PPART, PI], bf16)
                nc.gpsimd.tensor_tensor(DIOU[:, 0:Lk], OMI[:, 0:Lk],
                                        QD[:, 0:Lk], Alu.add)
                # atan-diff: x = (tw*ph - pw*th)/(th*ph + tw*pw) on half-whs
                CC1 = pW.tile([PPART, PI], bf16)
                CC2 = pW.tile([PPART, PI], bf16)
                CC3 = pW.tile([PPART, PI], bf16)
                CC4 = pW.tile([PPART, PI], bf16)
                tw = twhh2[:, 0, 0:Lk]
                th = twhh2[:, 1, 0:Lk]
                pw_ = geo[:, 0, 4, 0:Lk]
                ph_ = geo[:, 0, 5, 0:Lk]
                nc.vector.tensor_tensor(CC1[:, 0:Lk], tw, ph_, Alu.mult)
                nc.vector.tensor_tensor(CC2[:, 0:Lk], pw_, th, Alu.mult)
                nc.gpsimd.tensor_tensor(CC3[:, 0:Lk], th, ph_, Alu.mult)
                nc.gpsimd.tensor_tensor(CC4[:, 0:Lk], tw, pw_, Alu.mult)
                nc.vector.tensor_tensor(CC1[:, 0:Lk], CC1[:, 0:Lk],
                                        CC2[:, 0:Lk], Alu.subtract)  # num
                DENA = pW.tile([PPART, PI], f32)
                nc.vector.tensor_tensor(DENA[:, 0:Lk], CC3[:, 0:Lk],
                                        CC4[:, 0:Lk], Alu.add)
                nc.vector.reciprocal_approx_fast(DENA[:, 0:Lk], DENA[:, 0:Lk])
                X = pW.tile([PPART, PI], bf16)
                nc.vector.tensor_tensor(X[:, 0:Lk], CC1[:, 0:Lk],
                                        DENA[:, 0:Lk], Alu.mult)
                nc.scalar.activation(X[:, 0:Lk], X[:, 0:Lk], Act.Arctan)
                nc.scalar.activation(X[:, 0:Lk], X[:, 0:Lk], Act.Square,
                                     scale=float(2.0 / np.pi))  # X <- v
                ADEN = pW.tile([PPART, PI], f32)
                nc.vector.scalar_tensor_tensor(ADEN[:, 0:Lk], X[:, 0:Lk], EPS,
                                               OMI[:, 0:Lk], Alu.add, Alu.add)
                nc.vector.reciprocal_approx_fast(ADEN[:, 0:Lk], ADEN[:, 0:Lk])
                ALPHA = pW.tile([PPART, PI], bf16)
                nc.vector.tensor_tensor(ALPHA[:, 0:Lk], X[:, 0:Lk],
                                        ADEN[:, 0:Lk], Alu.mult)
                nc.vector.tensor_tensor(ALPHA[:, 0:Lk], ALPHA[:, 0:Lk],
                                        X[:, 0:Lk], Alu.mult)  # alpha*v
                nc.gpsimd.tensor_tensor(DIOU[:, 0:Lk], DIOU[:, 0:Lk],
                                        ALPHA[:, 0:Lk], Alu.add)  # cioup
                nc.vector.scalar_tensor_tensor(CC2[:, 0:Lk], DIOU[:, 0:Lk],
                                               1.0, Mm[:, 0:Lk], Alu.mult,
                                               Alu.mult,
                                               accum_out=ACC[:, cb + 2:cb + 3])
                if level < 4:
                    continue

                # ---- bce ----
                Z = pW.tile([PPART, C11 * PI], f16)
                z = Z[:].rearrange("p (c j) -> p c j", c=C11)
                nc.gpsimd.tensor_scalar(z[:, :, 0:Lk], selp[:, :, 0:Lk], -1.0,
                                        1.0, Alu.mult, Alu.add)
                nc.vector.tensor_scalar(z[:, :, 0:Lk], z[:, :, 0:Lk], 1e-4,
                                        None, Alu.max)
                t11i = T[:].bitcast(i16).rearrange("p (j c) -> p c j",
                                                   c=CH)[:, 4:CH, :]
                nc.vector.copy_predicated(z[:, :, 0:Lk], t11i,
                                          selp[:, :, 0:Lk])
                invi = INVM[:, 0:Lk].bitcast(i16)
                invb = invi.unsqueeze(1).broadcast_to([PPART, C11, Lk])
                onesb = ONES[:, 0:1].unsqueeze(1).broadcast_to(
                    [PPART, C11, Lk])
                nc.vector.copy_predicated(z[:, :, 0:Lk], invb, onesb)
                if act_accum:
                    nc.scalar.activation(z[:, 0:1, 0:Lk], z[:, 0:1, 0:Lk],
                                         Act.Ln, accum_out=ACC[:, cb + 1:cb + 2])
                    nc.scalar.activation(z[:, 1:C11, 0:Lk], z[:, 1:C11, 0:Lk],
                                         Act.Ln, accum_out=ACC[:, cb:cb + 1])
                else:
                    nc.scalar.activation(z[:, :, 0:Lk], z[:, :, 0:Lk], Act.Ln)
                    mb = Mm[:, 0:Lk].unsqueeze(1).broadcast_to(
                        [PPART, C11, Lk])
                    DUM2 = pW.tile([PPART, C11 * PI], f16)
                    d2 = DUM2[:].rearrange("p (c j) -> p c j", c=C11)
                    nc.vector.scalar_tensor_tensor(
                        d2[:, 1:C11, 0:Lk], z[:, 1:C11, 0:Lk], 1.0,
                        mb[:, 1:C11, :], Alu.mult, Alu.mult,
                        accum_out=ACC[:, cb:cb + 1])
                    nc.vector.scalar_tensor_tensor(
                        d2[:, 0:1, 0:Lk], z[:, 0:1, 0:Lk], 1.0, mb[:, 0:1, :],
                        Alu.mult, Alu.mult, accum_out=ACC[:, cb + 1:cb + 2])

            nc.sync.dma_start(accO, ACC[:])

    nc.compile()
    return nc


_build_bass = _build  # test.py compatibility


def kernel(pred, target):
    pred = np.ascontiguousarray(np.asarray(pred, dtype=np.float32))
    target = np.ascontiguousarray(np.asarray(target, dtype=np.float32))
    assert pred.shape == (B, A, N, CH) and target.shape == (B, N, CH)

    if "nc" not in _CACHE:
        _CACHE["nc"] = _build()
    nc = _CACHE["nc"]

    from concourse import bass_utils

    in_maps = []
    for c in range(NCORES):
        lo, hi = c * BPC, (c + 1) * BPC
        in_maps.append({
            "predL": np.ascontiguousarray(pred[lo:hi]),
            "targL": np.ascontiguousarray(target[lo:hi]),
        })

    res = bass_utils.run_bass_kernel_spmd(nc, in_maps, core_ids=list(range(NCORES)))
    _CACHE["last_results"] = res

    loss_b = []
    for c in range(NCORES):
        acc = res.results[c]["acc_out"].astype(np.float32)   # [128, 8]
        ab = acc.reshape(BPC, SEC, 2, 4).sum(axis=(1, 2),
                                             dtype=np.float32)  # [8, 4]
        lnc, lno, ciou, cnt = ab[:, 0], ab[:, 1], ab[:, 2], ab[:, 3]
        loss_b.append(ciou / cnt - lno / cnt - 0.1 * lnc / cnt)
    loss_b = np.concatenate(loss_b).astype(np.float32)
    return np.float32(np.mean(loss_b, dtype=np.float32))
